# revision 10
# baseline (speedup 1.0000x reference)
"""Trainium2 Bass kernel for nn_DiscriminatorBlock (StyleGAN2 discriminator
block): 3x3 conv + lrelu, FIR-blur + 3x3 stride-2 conv + lrelu, residual
FIR-blur + 1x1 stride-2 conv, merged with sqrt(0.5).

Strategy (per core, batch N=8 sharded over 8 cores, 1 image each):
- "mod-4 class" channel-major layout: stack[32*g + c, u] = x[4u + g, c]
  built via strided DMA loads + DVE 32x32 block transposes. Conv taps
  become contiguous partition-group windows + free-dim (u) shifts.
- Slab rows have pitch 160 = 5 x 32-col blocks: 128 data cols + 32-col
  zero guard, so one chunk transposes as a single contiguous 2D call and
  window reads at u=-1 / u=128 land in zero guards (SAME/blur padding).
- FIR blurs folded into the convs on the host (blur o conv = one 6x6 /
  4x4 stride-2 conv); eq-lr scales, sqrt(2) gains and sqrt(0.5) merge
  scale folded into weights.
- fp16 matmuls (fp32 PSUM accumulate), output classes col-packed via
  tile_position=(0, 32r).
"""
import sys
import os
sys.path.insert(0, '/opt/trn_rl_repo')
import numpy as np

H, W, C, F = 512, 512, 32, 32
N_CORES = 8
U = W // 4            # u-columns per image row (128)
RP = 160              # slab row pitch (128 data + 32 guard cols)
RB = 8                # image rows per load/transpose chunk

SQ2 = float(np.sqrt(2.0))
SQH = float(np.sqrt(0.5))


# ---------------------------------------------------------------- specs ----
def _mm_specs():
    """List of MM specs in canonical order: dict(conv, dyi, rpp, du, g0, g1,
    n) with n = weight tile index. conv0: S=1, D=[-1,0,1], 3 dys;
    conv1: S=2, D=[-2..3], 6 dys;  res: S=2, D=[-1..2], 4 dys."""
    conv0_r = {0: [(-1, 3, 4), (0, 0, 2)], 1: [(0, 0, 3)],
               2: [(0, 1, 4)], 3: [(0, 2, 4), (1, 0, 1)]}
    conv1_r = {0: [(-1, 2, 4), (0, 0, 4)],
               1: [(0, 0, 4), (1, 0, 2)],
               2: [(0, 2, 4), (1, 0, 4)],
               3: [(1, 0, 4), (2, 0, 2)]}
    res_r = {0: [(-1, 3, 4), (0, 0, 3)],
             1: [(0, 1, 4), (1, 0, 1)],
             2: [(0, 3, 4), (1, 0, 3)],
             3: [(1, 1, 4), (2, 0, 1)]}
    def _split(g0, g1):
        # SBUF APs from base partition 32g0 cannot cross the 64-partition
        # boundary unless g0 in {0, 2}; split [1, >2) into [1,2) + [2, ...).
        if g0 == 1 and g1 > 2:
            return [(1, 2), (2, g1)]
        return [(g0, g1)]

    out = []
    n = 0
    for conv, rspec, ndy in (("c0", conv0_r, 3), ("c1", conv1_r, 6),
                             ("rs", res_r, 4)):
        for dyi in range(ndy):
            for rpp in range(4):
                for (du, g0, g1) in rspec[rpp]:
                    for (a0, a1) in _split(g0, g1):
                        out.append(dict(conv=conv, dyi=dyi, rpp=rpp,
                                        du=du, g0=a0, g1=a1, n=n))
                        n += 1
    return out


_SPECS = _mm_specs()
NW = len(_SPECS)
_CONV_PARAMS = {"c0": dict(S=1, D=[-1, 0, 1]),
                "c1": dict(S=2, D=[-2, -1, 0, 1, 2, 3]),
                "rs": dict(S=2, D=[-1, 0, 1, 2])}


def _pack_host(w0, b0, w1, b1, w_res):
    """Fold eq-lr scales, FIR blur, and gains into per-MM fp16 weight tiles.
    Returns (wts [128, NW*32] fp16, biases [128, 2] fp32)."""
    bk = np.array([1.0, 3.0, 3.0, 1.0]) / 8.0
    W0e = w0.astype(np.float64) * (1.0 / np.sqrt(9 * 32))
    W1e = w1.astype(np.float64) * (1.0 / np.sqrt(9 * 32))
    WRe = w_res.astype(np.float64) * (1.0 / np.sqrt(1 * 32))
    K6 = np.zeros((6, 6, 32, 32))
    for r in range(3):
        for s in range(3):
            for a in range(4):
                for c in range(4):
                    K6[r + a, s + c] += bk[a] * bk[c] * W1e[r, s]
    K6 *= SQ2
    K4 = np.einsum('a,c,ij->acij', bk, bk, WRe[0, 0]) * SQH
    Wfull = {"c0": W0e, "c1": K6, "rs": K4}

    wts = np.zeros((128, NW * 32), np.float16)
    for sp in _SPECS:
        P = _CONV_PARAMS[sp["conv"]]
        Wf = Wfull[sp["conv"]]
        for g in range(sp["g0"], sp["g1"]):
            d = 4 * sp["du"] + g - P["S"] * sp["rpp"]
            di = P["D"].index(d)
            wts[32 * g:32 * g + 32, 32 * sp["n"]:32 * sp["n"] + 32] = \
                Wf[sp["dyi"], di].astype(np.float16)
    biases = np.zeros((128, 2), np.float32)
    biases[:, 0] = np.tile(b0.astype(np.float32).reshape(-1), 4)
    biases[:, 1] = np.tile(b1.astype(np.float32).reshape(-1), 4)
    return wts, biases


# ---------------------------------------------------------------- build ----
def _build(Himg=H, SO=32):
    import concourse.bass as bass
    import concourse.mybir as mybir
    from concourse import bacc
    from concourse.tile import TileContext

    F32 = mybir.dt.float32
    F16 = mybir.dt.float16
    ACTF = mybir.ActivationFunctionType

    HO = Himg // 2
    T = HO // SO                  # number of slabs
    assert T * SO == HO
    LX = 2 * SO + 6               # x-slab rows
    LXA = ((LX + RB - 1) // RB) * RB
    NCH = LXA // RB
    LH = 2 * SO + 4               # h0-slab rows
    NG0 = LH // 4                 # conv0 4-row groups
    NGO = SO // 8                 # out 8-row groups

    nc = bacc.Bacc('TRN2', target_bir_lowering=False)
    x_t = nc.dram_tensor("x", [Himg, W, C], F32, kind="ExternalInput")
    w_t = nc.dram_tensor("wts", [128, NW * 32], F16, kind="ExternalInput")
    b_t = nc.dram_tensor("biases", [128, 2], F32, kind="ExternalInput")
    o_t = nc.dram_tensor("out", [HO, W // 2, F], F32, kind="ExternalOutput")

    c0_specs = [sp for sp in _SPECS if sp["conv"] == "c0"]
    c1_specs = [sp for sp in _SPECS if sp["conv"] == "c1"]
    rs_specs = [sp for sp in _SPECS if sp["conv"] == "rs"]

    with TileContext(nc) as tc:
        with tc.tile_pool(name="const", bufs=1) as cpool, \
             tc.tile_pool(name="slab", bufs=2) as spool, \
             tc.tile_pool(name="stage", bufs=3) as gpool, \
             tc.tile_pool(name="psum", bufs=2, space="PSUM") as ppool:

            wtile = cpool.tile([128, NW * 32], F16, tag="wts")
            nc.sync.dma_start(out=wtile[:, :], in_=w_t[:, :])
            btile = cpool.tile([128, 2], F32, tag="bias")
            nc.sync.dma_start(out=btile[:, :], in_=b_t[:, :])

            TSX = 32 + LXA * RP + 8   # x-slab tile cols
            TSH = 32 + LH * RP + 8    # h0-slab tile cols

            for t in range(T):
                y0 = t * SO               # out-row base
                xb = 2 * y0 - 3           # image row of x-slab row 0
                hb = 2 * y0 - 2           # image row of h0-slab row 0

                X4 = spool.tile([128, TSX], F16, tag="X4")
                H0 = spool.tile([128, TSH], F16, tag="H0")
                # shifted row views: Xd[du][p, s, u] = flat 32+du + s*RP + u
                Xd = {du: X4[:, 32 + du:32 + du + LXA * RP].rearrange(
                    "p (s u) -> p s u", u=RP) for du in (-1, 0, 1, 2)}
                Hd = {du: H0[:, 32 + du:32 + du + LH * RP].rearrange(
                    "p (s u) -> p s u", u=RP) for du in (-1, 0, 1, 2)}

                # guards: leading block, tail pads, per-row guard blocks
                nc.gpsimd.memset(X4[:, 0:32], 0.0)
                nc.gpsimd.memset(X4[:, 32 + LXA * RP:], 0.0)
                nc.gpsimd.memset(Xd[0][:, :, 128:160], 0.0)
                nc.gpsimd.memset(H0[:, 0:32], 0.0)
                nc.gpsimd.memset(H0[:, 32 + LH * RP:], 0.0)
                nc.gpsimd.memset(Hd[0][:, :, 128:160], 0.0)
                if t == 0 or t == T - 1:
                    nc.gpsimd.memset(H0[:, :], 0.0)

                # ---- load + transpose chunks ----
                for ch in range(NCH):
                    nat0 = gpool.tile([128, RB * 128], F16, tag="nat0")
                    r0 = xb + ch * RB     # image row of chunk row 0
                    v0 = max(0, -r0)
                    v1 = min(RB, Himg - r0)
                    v1 = max(v1, v0)
                    if v0 > 0:
                        nc.gpsimd.memset(nat0[:, 0:128 * v0], 0.0)
                    if v1 < RB:
                        nc.gpsimd.memset(nat0[:, 128 * v1:], 0.0)
                    if v1 > v0:
                        src = x_t[r0 + v0:r0 + v1, :, :].rearrange(
                            "y (b j r) c -> r j (y b) c", b=4, j=32, r=4)
                        for ri in range(4):
                            nc.gpsimd.dma_start(
                                out=nat0[32 * ri:32 * ri + 32,
                                         128 * v0:128 * v1],
                                in_=src[ri])
                    # 2D transpose, then strided copy into the guarded slab
                    tst = gpool.tile([128, RB * 128], F16, tag="tst")
                    nc.vector.transpose(tst[:, :], nat0[:, :])
                    nc.vector.tensor_copy(
                        Xd[0][:, ch * RB:(ch + 1) * RB, 0:128], tst[:, :])

                # ---- conv0: h0 slab in 4-row groups ----
                for q in range(NG0):
                    s = 4 * q             # h0-slab row base
                    w0v = max(0, -(hb + s))
                    w1v = min(4, Himg - (hb + s))
                    if w1v <= w0v:
                        continue
                    ps0 = ppool.tile([128, 512], F32, tag="ps0")
                    nsee = {r: 0 for r in range(4)}
                    tot = {r: sum(1 for sp in c0_specs if sp["rpp"] == r)
                           for r in range(4)}
                    for sp in c0_specs:
                        dy = sp["dyi"] - 1
                        g0, g1 = sp["g0"], sp["g1"]
                        sx = s + dy + 1   # x-slab row for h0 row s
                        rhs = Xd[sp["du"]][32 * g0:32 * g1, sx:sx + 4, 0:128]
                        lhsT = wtile[32 * g0:32 * g1,
                                     32 * sp["n"]:32 * sp["n"] + 32]
                        r = sp["rpp"]
                        nsee[r] += 1
                        nc.tensor.matmul(
                            ps0[32 * r:32 * r + 32, :], lhsT, rhs,
                            start=(nsee[r] == 1), stop=(nsee[r] == tot[r]),
                            tile_position=(32 * g0, 32 * r),
                            skip_group_check=True)
                    ps0v = ps0[:, :].rearrange("p (a b) -> p a b", b=128)
                    nc.scalar.activation(
                        Hd[0][:, s + w0v:s + w1v, 0:128],
                        ps0v[:, w0v:w1v, :],
                        ACTF.Prelu, bias=btile[:, 0:1], alpha=0.2)

                # ---- conv1 + res per 8-out-row group ----
                for gq in range(NGO):
                    ps1 = ppool.tile([128, 512], F32, tag="ps1")
                    nsee = {r: 0 for r in range(4)}
                    tot = {r: sum(1 for sp in c1_specs if sp["rpp"] == r)
                           for r in range(4)}
                    for sp in c1_specs:
                        p = sp["dyi"]
                        g0, g1 = sp["g0"], sp["g1"]
                        s0r = 16 * gq + p     # h0-slab row: 2*yl + p
                        rhs = Hd[sp["du"]][32 * g0:32 * g1,
                                           s0r:s0r + 15:2, 0:127:2]
                        lhsT = wtile[32 * g0:32 * g1,
                                     32 * sp["n"]:32 * sp["n"] + 32]
                        r = sp["rpp"]
                        nsee[r] += 1
                        nc.tensor.matmul(
                            ps1[32 * r:32 * r + 32, :], lhsT, rhs,
                            start=(nsee[r] == 1), stop=(nsee[r] == tot[r]),
                            tile_position=(32 * g0, 32 * r),
                            skip_group_check=True)
                    h1sb = gpool.tile([128, 512], F32, tag="h1sb")
                    nc.scalar.activation(h1sb[:, :], ps1[:, :],
                                         ACTF.Prelu, bias=btile[:, 1:2],
                                         alpha=0.2)

                    ps2 = ppool.tile([128, 512], F32, tag="ps2")
                    nsee = {r: 0 for r in range(4)}
                    tot = {r: sum(1 for sp in rs_specs if sp["rpp"] == r)
                           for r in range(4)}
                    for sp in rs_specs:
                        a = sp["dyi"]
                        g0, g1 = sp["g0"], sp["g1"]
                        s0r = 16 * gq + a + 2   # x-slab row: 2*yl + a + 2
                        rhs = Xd[sp["du"]][32 * g0:32 * g1,
                                           s0r:s0r + 15:2, 0:127:2]
                        lhsT = wtile[32 * g0:32 * g1,
                                     32 * sp["n"]:32 * sp["n"] + 32]
                        r = sp["rpp"]
                        nsee[r] += 1
                        nc.tensor.matmul(
                            ps2[32 * r:32 * r + 32, :], lhsT, rhs,
                            start=(nsee[r] == 1), stop=(nsee[r] == tot[r]),
                            tile_position=(32 * g0, 32 * r),
                            skip_group_check=True)

                    osum = gpool.tile([128, 512], F32, tag="osum")
                    nc.vector.tensor_add(osum[:, :], h1sb[:, :], ps2[:, :])
                    onat = gpool.tile([128, 512], F32, tag="onat")
                    nc.vector.transpose(onat[:, :], osum[:, :])
                    # store: out[y0+8gq+yl, 128k+4ul+rpp, f]
                    dst = o_t[y0 + 8 * gq:y0 + 8 * gq + 8, :, :].rearrange(
                        "y (k u r) f -> r u y k f", k=2, u=32, r=4)
                    for rpp in range(4):
                        srcp = onat[32 * rpp:32 * rpp + 32, :].rearrange(
                            "p (y k f) -> p y k f", k=2, f=32)
                        nc.sync.dma_start(out=dst[rpp], in_=srcp)

    nc.compile()
    return nc


# ----------------------------------------------------------------- run ----
_CACHE = {}
LAST_RESULTS = None


def _get_nc(Himg=H, SO=32):
    key = (Himg, SO)
    if key not in _CACHE:
        _CACHE[key] = _build(Himg, SO)
    return _CACHE[key]


def kernel(x, w0, b0, w1, b1, w_res):
    from concourse.bass_utils import run_bass_kernel_spmd
    x = np.asarray(x, np.float32)
    wts, biases = _pack_host(np.asarray(w0), np.asarray(b0), np.asarray(w1),
                             np.asarray(b1), np.asarray(w_res))
    nc = _get_nc(H, 32)
    in_maps = [{"x": np.ascontiguousarray(x[i]), "wts": wts,
                "biases": biases} for i in range(N_CORES)]
    res = run_bass_kernel_spmd(nc, in_maps, core_ids=list(range(N_CORES)))
    global LAST_RESULTS
    LAST_RESULTS = res
    out = np.stack([res.results[i]["out"] for i in range(N_CORES)])
    return out.astype(np.float32)


# revision 12
# speedup vs baseline: 3.0234x; 3.0234x over previous
"""Trainium2 Bass kernel for nn_DiscriminatorBlock (StyleGAN2 discriminator
block): 3x3 conv + lrelu, FIR-blur + 3x3 stride-2 conv + lrelu, residual
FIR-blur + 1x1 stride-2 conv, merged with sqrt(0.5).

Per core (batch 8 sharded over 8 cores): "mod-4 class" channel-major
layout stack[32g + c, u] = x[4u + g, c] built via strided DMA + DVE 32x32
transposes; slab rows pitch 160 (128 data + 32-col zero guard) so window
reads at u=-1/128 hit zeros. FIR blurs folded into the convs on the host
(6x6 / 4x4 stride-2 fused kernels, all scales folded). fp16 matmuls with
dense [K<=128, M<=128] zero-padded weight blocks covering all 4 output
classes per instruction; fp32 PSUM; Prelu(alpha=0.2) on ACT. Slab loads
are emitted one slab ahead so DMA/DVE overlap PE compute.
"""
import sys
import os
sys.path.insert(0, '/opt/trn_rl_repo')
import numpy as np

H, W, C, F = 512, 512, 32, 32
N_CORES = 8
U = W // 4
RP = 160              # slab row pitch (128 data + 32 guard cols)
RB = 8                # image rows per load/transpose chunk

SQ2 = float(np.sqrt(2.0))
SQH = float(np.sqrt(0.5))

# dense MM blocks: (du, g0, g1, m0, m1); first n_start entries carry
# start=True on the first dy (they exactly tile the M ranges they start).
_DENSE = {
    "c0": dict(S=1, D=[-1, 0, 1], dys=[-1, 0, 1], n_start=1,
               mms=[(0, 0, 4, 0, 4), (-1, 3, 4, 0, 1), (1, 0, 1, 3, 4)]),
    "c1": dict(S=2, D=[-2, -1, 0, 1, 2, 3], dys=[p - 2 for p in range(6)],
               n_start=2,
               mms=[(0, 0, 4, 0, 3), (2, 0, 2, 3, 4), (-1, 2, 4, 0, 1),
                    (1, 0, 2, 1, 2), (1, 0, 4, 2, 4)]),
    "rs": dict(S=2, D=[-1, 0, 1, 2], dys=[a - 1 for a in range(4)],
               n_start=2,
               mms=[(0, 0, 4, 0, 3), (2, 0, 1, 3, 4), (-1, 3, 4, 0, 1),
                    (1, 0, 1, 1, 2), (1, 0, 4, 2, 4)]),
}


def _specs():
    """Flatten to per-conv lists of dicts with weight-tile col offsets."""
    out = {}
    off = 0
    for conv in ("c0", "c1", "rs"):
        P = _DENSE[conv]
        lst = []
        for dyi in range(len(P["dys"])):
            for j, (du, g0, g1, m0, m1) in enumerate(P["mms"]):
                w = 32 * (m1 - m0)
                lst.append(dict(dyi=dyi, du=du, g0=g0, g1=g1, m0=m0, m1=m1,
                                off=off, wid=w,
                                start=(dyi == 0 and j < P["n_start"]),
                                stop=(dyi == len(P["dys"]) - 1)))
                off += w
        out[conv] = lst
    return out, off


_SPECS, _WCOLS = _specs()


def _pack_host(w0, b0, w1, b1, w_res):
    bk = np.array([1.0, 3.0, 3.0, 1.0]) / 8.0
    W0e = w0.astype(np.float64) * (1.0 / np.sqrt(9 * 32))
    W1e = w1.astype(np.float64) * (1.0 / np.sqrt(9 * 32))
    WRe = w_res.astype(np.float64) * (1.0 / np.sqrt(1 * 32))
    K6 = np.zeros((6, 6, 32, 32))
    for r in range(3):
        for s in range(3):
            for a in range(4):
                for c in range(4):
                    K6[r + a, s + c] += bk[a] * bk[c] * W1e[r, s]
    K6 *= SQ2
    K4 = np.einsum('a,c,ij->acij', bk, bk, WRe[0, 0]) * SQH
    Wfull = {"c0": W0e, "c1": K6, "rs": K4}

    wts = np.zeros((128, _WCOLS), np.float16)
    for conv in ("c0", "c1", "rs"):
        P = _DENSE[conv]
        Wf = Wfull[conv]
        for sp in _SPECS[conv]:
            for g in range(sp["g0"], sp["g1"]):
                for r in range(sp["m0"], sp["m1"]):
                    d = 4 * sp["du"] + g - P["S"] * r
                    if d not in P["D"]:
                        continue
                    di = P["D"].index(d)
                    wts[32 * g:32 * g + 32,
                        sp["off"] + 32 * (r - sp["m0"]):
                        sp["off"] + 32 * (r - sp["m0"]) + 32] = \
                        Wf[sp["dyi"], di].astype(np.float16)
    biases = np.zeros((128, 2), np.float32)
    biases[:, 0] = np.tile(b0.astype(np.float32).reshape(-1), 4)
    biases[:, 1] = np.tile(b1.astype(np.float32).reshape(-1), 4)
    return wts, biases


def _build(Himg=H, SO=32):
    import concourse.mybir as mybir
    from concourse import bacc
    from concourse.tile import TileContext

    F32 = mybir.dt.float32
    F16 = mybir.dt.float16
    ACTF = mybir.ActivationFunctionType

    HO = Himg // 2
    T = HO // SO
    assert T * SO == HO
    LX = 2 * SO + 6
    LXA = ((LX + RB - 1) // RB) * RB
    NCH = LXA // RB
    LH = 2 * SO + 4
    NG0 = LH // 4
    NGO = SO // 8

    nc = bacc.Bacc('TRN2', target_bir_lowering=False)
    x_t = nc.dram_tensor("x", [Himg, W, C], F32, kind="ExternalInput")
    w_t = nc.dram_tensor("wts", [128, _WCOLS], F16, kind="ExternalInput")
    b_t = nc.dram_tensor("biases", [128, 2], F32, kind="ExternalInput")
    o_t = nc.dram_tensor("out", [HO, W // 2, F], F32, kind="ExternalOutput")

    TSX = 32 + LXA * RP + 8
    TSH = 32 + LH * RP + 8

    with TileContext(nc) as tc:
        with tc.tile_pool(name="const", bufs=1) as cpool, \
             tc.tile_pool(name="slab", bufs=2) as spool, \
             tc.tile_pool(name="stage", bufs=3) as gpool, \
             tc.tile_pool(name="psum", bufs=2, space="PSUM") as ppool:

            wtile = cpool.tile([128, _WCOLS], F16, tag="wts")
            nc.sync.dma_start(out=wtile[:, :], in_=w_t[:, :])
            btile = cpool.tile([128, 2], F32, tag="bias")
            nc.sync.dma_start(out=btile[:, :], in_=b_t[:, :])

            slabs = {}

            def emit_load(t):
                y0 = t * SO
                xb = 2 * y0 - 3
                X4 = spool.tile([128, TSX], F16, tag="X4")
                H0 = spool.tile([128, TSH], F16, tag="H0")
                Xd = {du: X4[:, 32 + du:32 + du + LXA * RP].rearrange(
                    "p (s u) -> p s u", u=RP) for du in (-1, 0, 1, 2)}
                Hd = {du: H0[:, 32 + du:32 + du + LH * RP].rearrange(
                    "p (s u) -> p s u", u=RP) for du in (-1, 0, 1, 2)}
                slabs[t] = (X4, H0, Xd, Hd)

                nc.gpsimd.memset(X4[:, 0:32], 0.0)
                nc.gpsimd.memset(X4[:, 32 + LXA * RP:], 0.0)
                nc.gpsimd.memset(Xd[0][:, :, 128:160], 0.0)
                nc.gpsimd.memset(H0[:, 0:32], 0.0)
                nc.gpsimd.memset(H0[:, 32 + LH * RP:], 0.0)
                nc.gpsimd.memset(Hd[0][:, :, 128:160], 0.0)
                if t == 0 or t == T - 1:
                    nc.gpsimd.memset(H0[:, :], 0.0)

                for ch in range(NCH):
                    nat0 = gpool.tile([128, RB * 128], F16, tag="nat0")
                    r0 = xb + ch * RB
                    v0 = max(0, -r0)
                    v1 = max(min(RB, Himg - r0), v0)
                    if v0 > 0:
                        nc.gpsimd.memset(nat0[:, 0:128 * v0], 0.0)
                    if v1 < RB:
                        nc.gpsimd.memset(nat0[:, 128 * v1:], 0.0)
                    if v1 > v0:
                        src = x_t[r0 + v0:r0 + v1, :, :].rearrange(
                            "y (b j r) c -> r j (y b) c", b=4, j=32, r=4)
                        for ri in range(4):
                            nc.gpsimd.dma_start(
                                out=nat0[32 * ri:32 * ri + 32,
                                         128 * v0:128 * v1],
                                in_=src[ri])
                    tst = gpool.tile([128, RB * 128], F16, tag="tst")
                    nc.vector.transpose(tst[:, :], nat0[:, :])
                    nc.vector.tensor_copy(
                        Xd[0][:, ch * RB:(ch + 1) * RB, 0:128], tst[:, :])

            def emit_mm(psum, wtile, src_v, sp, row0, rstep, cstep):
                """One dense MM: rhs rows from src_v at row0 (step rstep),
                cols step cstep."""
                g0, g1 = sp["g0"], sp["g1"]
                if cstep == 1:
                    rhs = src_v[sp["du"]][32 * g0:32 * g1,
                                          row0:row0 + 4, 0:128]
                else:
                    rhs = src_v[sp["du"]][32 * g0:32 * g1,
                                          row0:row0 + 15:2, 0:127:2]
                lhsT = wtile[32 * g0:32 * g1, sp["off"]:sp["off"] + sp["wid"]]
                nc.tensor.matmul(
                    psum[32 * sp["m0"]:32 * sp["m1"], :], lhsT, rhs,
                    start=sp["start"], stop=sp["stop"],
                    tile_position=(32 * g0, 32 * sp["m0"]),
                    skip_group_check=True)

            def emit_conv0(t):
                y0 = t * SO
                hb = 2 * y0 - 2
                X4, H0, Xd, Hd = slabs[t]
                for q in range(NG0):
                    s = 4 * q
                    w0v = max(0, -(hb + s))
                    w1v = min(4, Himg - (hb + s))
                    if w1v <= w0v:
                        continue
                    ps0 = ppool.tile([128, 512], F32, tag="ps0")
                    for sp in _SPECS["c0"]:
                        dy = _DENSE["c0"]["dys"][sp["dyi"]]
                        emit_mm(ps0, wtile, Xd, sp, s + dy + 1, 1, 1)
                    ps0v = ps0[:, :].rearrange("p (a b) -> p a b", b=128)
                    nc.scalar.activation(
                        Hd[0][:, s + w0v:s + w1v, 0:128],
                        ps0v[:, w0v:w1v, :],
                        ACTF.Prelu, bias=btile[:, 0:1], alpha=0.2)

            def emit_c1rs(t):
                y0 = t * SO
                X4, H0, Xd, Hd = slabs[t]
                for gq in range(NGO):
                    ps1 = ppool.tile([128, 512], F32, tag="ps1")
                    for sp in _SPECS["c1"]:
                        p = _DENSE["c1"]["dys"][sp["dyi"]] + 2
                        emit_mm(ps1, wtile, Hd, sp, 16 * gq + p, 2, 2)
                    h1sb = gpool.tile([128, 512], F32, tag="h1sb")
                    nc.scalar.activation(h1sb[:, :], ps1[:, :], ACTF.Prelu,
                                         bias=btile[:, 1:2], alpha=0.2)
                    ps2 = ppool.tile([128, 512], F32, tag="ps2")
                    for sp in _SPECS["rs"]:
                        a = _DENSE["rs"]["dys"][sp["dyi"]] + 1
                        emit_mm(ps2, wtile, Xd, sp, 16 * gq + a + 2, 2, 2)
                    osum = gpool.tile([128, 512], F32, tag="osum")
                    nc.vector.tensor_add(osum[:, :], h1sb[:, :], ps2[:, :])
                    onat = gpool.tile([128, 512], F32, tag="onat")
                    nc.vector.transpose(onat[:, :], osum[:, :])
                    dst = o_t[y0 + 8 * gq:y0 + 8 * gq + 8, :, :].rearrange(
                        "y (k u r) f -> r u y k f", k=2, u=32, r=4)
                    for rpp in range(4):
                        srcp = onat[32 * rpp:32 * rpp + 32, :].rearrange(
                            "p (y k f) -> p y k f", k=2, f=32)
                        nc.sync.dma_start(out=dst[rpp], in_=srcp)

            emit_load(0)
            for t in range(T):
                if t + 1 < T:
                    emit_load(t + 1)
                emit_conv0(t)
                emit_c1rs(t)
                del slabs[t]

    nc.compile()
    return nc


_CACHE = {}
LAST_RESULTS = None


def _get_nc(Himg=H, SO=32):
    key = (Himg, SO)
    if key not in _CACHE:
        _CACHE[key] = _build(Himg, SO)
    return _CACHE[key]


def kernel(x, w0, b0, w1, b1, w_res):
    from concourse.bass_utils import run_bass_kernel_spmd
    x = np.asarray(x, np.float32)
    wts, biases = _pack_host(np.asarray(w0), np.asarray(b0), np.asarray(w1),
                             np.asarray(b1), np.asarray(w_res))
    nc = _get_nc(H, 32)
    in_maps = [{"x": np.ascontiguousarray(x[i]), "wts": wts,
                "biases": biases} for i in range(N_CORES)]
    res = run_bass_kernel_spmd(nc, in_maps, core_ids=list(range(N_CORES)))
    global LAST_RESULTS
    LAST_RESULTS = res
    out = np.stack([res.results[i]["out"] for i in range(N_CORES)])
    return out.astype(np.float32)
